# revision 1
# baseline (speedup 1.0000x reference)
"""Trainium2 Bass kernel for topk_masking (nn_CustomModule_8065948582484).

Reference semantics (per batch b):
  idx[b,f] = argmax(score[b,f,:196])                 (first index on ties)
  mask grows from a fixed prior region on a 14x14 grid; at frame f the
  argmax cell is added iff it is 4-adjacent to the current mask.
  out = [ones(B,1), masks frame-major] -> [B, 1+16*196] fp32.

Strategy (pure batch data-parallel across 8 cores, 2048 batches/core,
16 tiles of 128 batches on SBUF partitions):
  1. argmax via prefix-max scan with per-frame guard resets (DVE, 2 tiles
     per scan); idx = popcount(prefix_max < max), split between ScalarE
     (sign+accum) and DVE (is_lt+accum) to balance engines.
  2. r via rowend-prefix popcount, c = idx - 14r.
  3. per-frame "added" recurrence on a 16-node adjacency graph (GS=8),
     plus first-hit dedup so each cell is scattered at most once.
  4. cov[p] = 17 - birth_frame(p) built by ONE gpsimd local_scatter per
     tile (negative index = not added -> ignored); prior merged via max.
     masks[f] = (cov >= 17-f) -> 16 tensor_scalar ops batched across
     4 tiles writing fp32 directly.
  5. fp32 output stored 4 tiles per HWDGE DMA.
"""

import sys

import numpy as np

for _p in ("/opt/trn_rl_repo",):
    if _p not in sys.path:
        sys.path.insert(0, _p)

from concourse import bacc, mybir, tile  # noqa: E402
from concourse.bass_utils import run_bass_kernel_spmd  # noqa: E402

B, F, P = 16384, 16, 196
N = 14  # grid side
S = P + 1  # slots per frame in the scan layout (guard + 196)
NCORES = 8
BLOC = B // NCORES  # 2048
NT = BLOC // 128  # 16 tiles per core
G2 = 2  # tiles per argmax scan / input DMA
G4 = 2  # tiles per output DMA
GS = 8  # tiles per batched phase-B group
NG = NT // GS
NF_DVE = 7  # frames of the idx-popcount done on DVE
NF_POOL = 0  # frames of the idx-popcount done on GPSIMD (rest on ScalarE)
GRAPH_ON_POOL = False  # pairwise graph builds on GPSIMD (crashed a device once)
SKIP_SCATTER = False  # debug: replace local_scatter with memset (wrong output)
SC_BUFS = 3  # scan-buffer depth

ALU = mybir.AluOpType
AX = mybir.AxisListType
F32 = mybir.dt.float32
BF16 = mybir.dt.bfloat16
I16 = mybir.dt.int16
ACT = mybir.ActivationFunctionType
BIG = 1e30


def build_nc():
    nc = bacc.Bacc(trn_type="TRN2", target_bir_lowering=False)
    score_d = nc.declare_dram_parameter("score", [BLOC, F, P], F32, isOutput=False)
    out_d = nc.declare_dram_parameter("out", [BLOC, 1 + F * P], F32, isOutput=True)

    with tile.TileContext(nc) as tc:
        with (
            tc.tile_pool(name="consts", bufs=1) as cpool,
            tc.tile_pool(name="scan", bufs=2) as spool,
            tc.tile_pool(name="masks", bufs=2) as mpool,
            tc.tile_pool(name="grp", bufs=2) as gpool,
        ):
            # ---- constants ----
            prior17 = cpool.tile([128, P], BF16, name="prior17")
            nc.vector.memset(prior17[:], 0.0)
            p17v = prior17.rearrange("q (r c) -> q r c", r=N)
            nc.vector.memset(p17v[:, 4:14, 2:12], 17.0)
            # scatter data: w[f] = 17 - f  (17, 16, ..., 2)
            wvals = cpool.tile([128, F], BF16, name="wvals")
            nc.gpsimd.iota(
                wvals[:], pattern=[[-1, F]], base=17, channel_multiplier=0,
                allow_small_or_imprecise_dtypes=True,
            )
            # strict lower-triangular [e,f] mask (e < f), bcast over partitions
            ltri = cpool.tile([128, F, F], BF16, name="ltri")
            nc.vector.memset(ltri[:], 0.0)
            for e in range(F - 1):
                nc.vector.memset(ltri[:, e, e + 1 : F], 1.0)
            d1 = cpool.tile([128, G2 * F * S], BF16, name="d1")
            nc.vector.memset(d1[:], BIG)
            d1v = d1.rearrange("q (a f s) -> q a f s", a=G2, f=F)
            nc.vector.memset(d1v[:, :, :, 0:1], -BIG)

            for g in range(NG):
                idxa = gpool.tile([128, F, GS], F32, tag="idxa", name="idxa")

                # ---- phase A: load / scan / popcounts, 2 tiles at a time ----
                for s in range(GS // G2):
                    r0 = (g * GS + s * G2) * 128
                    sc = spool.tile(
                        [128, G2 * F * S], F32, tag="sc", name="sc", bufs=SC_BUFS
                    )
                    scv = sc.rearrange("q (a f s) -> q a f s", a=G2, f=F)
                    nc.vector.memset(scv[:, :, :, 0:1], -BIG)
                    for j in range(G2):
                        nc.sync.dma_start(
                            out=scv[:, j, :, 1:S],
                            in_=score_d[r0 + j * 128 : r0 + (j + 1) * 128],
                        )
                    # in-place prefix-max scan with guard resets
                    nc.vector.tensor_tensor_scan(
                        sc[:], sc[:], d1[:], 0.0, ALU.max, ALU.min
                    )
                    # idx = #positions with prefix-max strictly below frame max
                    for j in range(G2):
                        t = s * G2 + j
                        for f in range(F):
                            if f < NF_DVE:
                                nc.vector.tensor_scalar(
                                    gpool.tile(
                                        [128, P], BF16, tag="vjunk",
                                        name="vjunk", bufs=2,
                                    ),
                                    scv[:, j, f, 1:S],
                                    scv[:, j, f, P : P + 1],
                                    None,
                                    ALU.is_lt,
                                    ALU.add,
                                    accum_out=idxa[:, f, t : t + 1],
                                )
                            else:
                                nc.scalar.activation(
                                    gpool.tile(
                                        [128, P], BF16, tag="sjunk",
                                        name="sjunk", bufs=2,
                                    ),
                                    scv[:, j, f, 1:S],
                                    ACT.Sign,
                                    bias=scv[:, j, f, P : P + 1],
                                    scale=-1.0,
                                    accum_out=idxa[:, f, t : t + 1],
                                )
                # ---- phase B: batched small compute for the whole group ----
                # r = round(idx/14 - 0.4643): value sits in [r-.46, r+.46],
                # so HW round-to-nearest int16 conversion recovers r exactly
                rq = gpool.tile([128, F, GS], F32, tag="rq", name="rq")
                nc.vector.tensor_scalar(
                    rq[:], idxa[:], 1.0 / 14.0, -0.4643, ALU.mult, ALU.add
                )
                ri = gpool.tile([128, F, GS], I16, tag="ri", name="ri")
                nc.vector.tensor_copy(ri[:], rq[:])
                rr = gpool.tile([128, F, GS], F32, tag="rr", name="rr")
                nc.vector.tensor_copy(rr[:], ri[:])
                cc = gpool.tile([128, F, GS], F32, tag="cc", name="cc")
                nc.vector.scalar_tensor_tensor(
                    cc[:], rr[:], -14.0, idxa[:], ALU.mult, ALU.add
                )
                vv = gpool.tile([128, F, GS], F32, tag="vv", name="vv")
                nc.vector.scalar_tensor_tensor(
                    vv[:], rr[:], 16.0, cc[:], ALU.mult, ALU.add
                )
                vb = gpool.tile([128, F, GS], BF16, tag="vb", name="vb")
                nc.vector.tensor_copy(vb[:], vv[:])

                # pairwise grid: dv[e,f,t] = v_e - v_f ; gg = adjacency
                ge = nc.gpsimd if GRAPH_ON_POOL else nc.vector
                dv = gpool.tile([128, F, F, GS], BF16, tag="dv", name="dv", bufs=1)
                ge.tensor_tensor(
                    dv[:],
                    vb.unsqueeze(2).broadcast_to([128, F, F, GS]),
                    vb.unsqueeze(1).broadcast_to([128, F, F, GS]),
                    ALU.subtract,
                )
                sq = gpool.tile([128, F, F, GS], BF16, tag="sq", name="sq", bufs=1)
                ge.tensor_tensor(sq[:], dv[:], dv[:], ALU.mult)
                g1 = gpool.tile([128, F, F, GS], BF16, tag="g1", name="g1", bufs=1)
                nc.vector.tensor_scalar(g1[:], sq[:], 1.0, None, ALU.is_equal)
                g16 = gpool.tile(
                    [128, F, F, GS], BF16, tag="g16", name="g16", bufs=1
                )
                nc.vector.tensor_scalar(g16[:], sq[:], 256.0, None, ALU.is_equal)
                gg = gpool.tile([128, F, F, GS], BF16, tag="gg", name="gg", bufs=1)
                ge.tensor_tensor(gg[:], g1[:], g16[:], ALU.add)
                # same-cell (for dedup): se[e,f,t] = (v_e == v_f) & (e < f)
                se = gpool.tile([128, F, F, GS], BF16, tag="se", name="se", bufs=1)
                nc.vector.scalar_tensor_tensor(
                    se[:],
                    sq[:],
                    0.0,
                    ltri.unsqueeze(3).broadcast_to([128, F, F, GS]),
                    ALU.is_equal,
                    ALU.mult,
                )

                # A = (r>=3 & 2<=c<=11) | (r>=4 & 1<=c<=12)
                u3 = gpool.tile([128, F, GS], BF16, tag="u3", name="u3")
                nc.vector.tensor_scalar(u3[:], rr[:], 3.0, None, ALU.is_ge)
                u4 = gpool.tile([128, F, GS], BF16, tag="u4", name="u4")
                nc.vector.tensor_scalar(u4[:], rr[:], 4.0, None, ALU.is_ge)
                cm2 = gpool.tile([128, F, GS], F32, tag="cm2", name="cm2")
                nc.vector.tensor_scalar(cm2[:], cc[:], 2.0, None, ALU.subtract)
                q1 = gpool.tile([128, F, GS], F32, tag="q1", name="q1")
                nc.vector.scalar_tensor_tensor(
                    q1[:], cc[:], -11.0, cm2[:], ALU.add, ALU.mult
                )
                b1 = gpool.tile([128, F, GS], BF16, tag="b1", name="b1")
                nc.vector.tensor_scalar(b1[:], q1[:], 0.0, None, ALU.is_le)
                cm1 = gpool.tile([128, F, GS], F32, tag="cm1", name="cm1")
                nc.vector.tensor_scalar(cm1[:], cc[:], 1.0, None, ALU.subtract)
                q2 = gpool.tile([128, F, GS], F32, tag="q2", name="q2")
                nc.vector.scalar_tensor_tensor(
                    q2[:], cc[:], -12.0, cm1[:], ALU.add, ALU.mult
                )
                b2 = gpool.tile([128, F, GS], BF16, tag="b2", name="b2")
                nc.vector.tensor_scalar(b2[:], q2[:], 0.0, None, ALU.is_le)
                t1 = gpool.tile([128, F, GS], BF16, tag="t1", name="t1")
                nc.vector.tensor_tensor(t1[:], u3[:], b1[:], ALU.logical_and)
                t2 = gpool.tile([128, F, GS], BF16, tag="t2", name="t2")
                nc.vector.tensor_tensor(t2[:], u4[:], b2[:], ALU.logical_and)
                aa = gpool.tile([128, F, GS], BF16, tag="aa", name="aa")
                nc.vector.tensor_tensor(aa[:], t1[:], t2[:], ALU.logical_or)

                # sequential added-recurrence:
                # added[f] = max(A[f], max_e added[e]*G[e,f])
                added = gpool.tile([128, F, GS], BF16, tag="added", name="added")
                nc.vector.memset(added[:], 0.0)
                t16 = gpool.tile([128, F, GS], BF16, tag="t16", name="t16")
                mx = gpool.tile([128, GS], F32, tag="mx", name="mx")
                for f in range(F):
                    nc.vector.tensor_tensor(
                        t16[:], added[:], gg[:, :, f, :], ALU.mult
                    )
                    t16v = t16.rearrange("q e t -> q t e")
                    nc.vector.tensor_reduce(mx[:], t16v, axis=AX.X, op=ALU.max)
                    nc.vector.tensor_tensor(
                        added[:, f, :], mx[:], aa[:, f, :], ALU.max
                    )

                # first-hit dedup: hb[f] = max_e added[e]*se[e,f]; fh = added & !hb
                hbt = gpool.tile(
                    [128, F, F, GS], BF16, tag="hbt", name="hbt", bufs=1
                )
                ge.tensor_tensor(
                    hbt[:],
                    added.unsqueeze(2).broadcast_to([128, F, F, GS]),
                    se[:],
                    ALU.mult,
                )
                hb = gpool.tile([128, F, GS], BF16, tag="hb", name="hb")
                hbtv = hbt.rearrange("q e f t -> q f t e")
                nc.vector.tensor_reduce(hb[:], hbtv, axis=AX.X, op=ALU.max)
                nhb = gpool.tile([128, F, GS], BF16, tag="nhb", name="nhb")
                nc.vector.tensor_scalar(nhb[:], hb[:], 0.0, None, ALU.is_equal)
                fh = gpool.tile([128, F, GS], BF16, tag="fh", name="fh")
                nc.vector.tensor_tensor(fh[:], added[:], nhb[:], ALU.mult)

                # scatter indices: idxs[f] = fh ? idx : -1, int16, t-major
                im0 = gpool.tile([128, F, GS], F32, tag="im0", name="im0")
                nc.vector.scalar_tensor_tensor(
                    im0[:], idxa[:], 1.0, fh[:], ALU.add, ALU.mult
                )
                idxm = gpool.tile([128, F, GS], F32, tag="idxm", name="idxm")
                nc.vector.tensor_scalar(idxm[:], im0[:], 1.0, None, ALU.subtract)
                idxs16 = gpool.tile([128, GS, F], I16, tag="idxs16", name="idxs16")
                nc.vector.tensor_copy(
                    idxs16[:], idxm.rearrange("q f t -> q t f")
                )

                # ---- phase C: scatter cov, compare-threshold, store 4 tiles ----
                for h in range(GS // G4):
                    r0 = (g * GS + h * G4) * 128
                    cov = gpool.tile([128, G4, P], BF16, tag="cov", name="cov")
                    for j in range(G4):
                        k = h * G4 + j
                        if SKIP_SCATTER:
                            nc.vector.memset(cov[:, j, :], 0.0)
                        else:
                            nc.gpsimd.local_scatter(
                                cov[:, j, :],
                                wvals[:],
                                idxs16[:, k, :],
                                channels=128,
                                num_elems=P,
                                num_idxs=F,
                            )
                    covm = gpool.tile([128, G4, P], BF16, tag="covm", name="covm")
                    nc.vector.tensor_tensor(
                        covm[:],
                        cov[:],
                        prior17.unsqueeze(1).broadcast_to([128, G4, P]),
                        ALU.max,
                    )
                    out_t = mpool.tile(
                        [128, G4, 1 + F * P], F32, tag="out_t", name="out_t"
                    )
                    nc.vector.memset(out_t[:, :, 0:1], 1.0)
                    for f in range(F):
                        nc.vector.tensor_scalar(
                            out_t[:, :, 1 + f * P : 1 + (f + 1) * P],
                            covm[:],
                            float(17 - f),
                            None,
                            ALU.is_ge,
                        )
                    nc.sync.dma_start(
                        out=out_d[r0 : r0 + G4 * 128].rearrange(
                            "(a p) w -> p a w", a=G4
                        ),
                        in_=out_t[:],
                    )

    nc.compile()
    return nc


_nc = None


def _get_nc():
    global _nc
    if _nc is None:
        _nc = build_nc()
    return _nc


def kernel(score, topn=196):
    score = np.ascontiguousarray(np.asarray(score, dtype=np.float32)).reshape(B, F, P)
    nc = _get_nc()
    in_maps = [
        {"score": score[i * BLOC : (i + 1) * BLOC]} for i in range(NCORES)
    ]
    res = run_bass_kernel_spmd(nc, in_maps, list(range(NCORES)))
    out = np.concatenate([res.results[i]["out"] for i in range(NCORES)], axis=0)
    return out



# revision 4
# speedup vs baseline: 6.6549x; 6.6549x over previous
"""Trainium2 Bass kernel for topk_masking (nn_CustomModule_8065948582484).

Reference semantics (per batch b):
  idx[b,f] = argmax(score[b,f,:196])                 (first index on ties)
  mask grows from a fixed prior region on a 14x14 grid; at frame f the
  argmax cell is added iff it is 4-adjacent to the current mask.
  out = [ones(B,1), masks frame-major] -> [B, 1+16*196] fp32.

Strategy (pure batch data-parallel across 8 cores, 2048 batches/core,
16 tiles of 128 batches on SBUF partitions):
  1. argmax via prefix-max scan with per-frame guard resets (DVE, 2 tiles
     per scan); idx = popcount(prefix_max < max), split between ScalarE
     (sign+accum) and DVE (is_lt+accum) to balance engines.
  2. r via rowend-prefix popcount, c = idx - 14r.
  3. per-frame "added" recurrence on a 16-node adjacency graph (GS=8),
     plus first-hit dedup so each cell is scattered at most once.
  4. cov[p] = 17 - birth_frame(p) built by ONE gpsimd local_scatter per
     tile (negative index = not added -> ignored); prior merged via max.
     masks[f] = (cov >= 17-f) -> 16 tensor_scalar ops batched across
     4 tiles writing fp32 directly.
  5. fp32 output stored 4 tiles per HWDGE DMA.
"""

import sys

import numpy as np

for _p in ("/opt/trn_rl_repo",):
    if _p not in sys.path:
        sys.path.insert(0, _p)

from concourse import bacc, mybir, tile  # noqa: E402
from concourse.bass_utils import run_bass_kernel_spmd  # noqa: E402

B, F, P = 16384, 16, 196
N = 14  # grid side
S = P + 1  # slots per frame in the scan layout (guard + 196)
NCORES = 8
BLOC = B // NCORES  # 2048
NT = BLOC // 128  # 16 tiles per core
G2 = 2  # tiles per argmax scan / input DMA
G4 = 2  # tiles per output DMA
GS = 8  # tiles per batched phase-B group
NG = NT // GS
NF_DVE = 7  # frames of the idx-popcount done on DVE
NF_POOL = 0  # frames of the idx-popcount done on GPSIMD (rest on ScalarE)
GRAPH_ON_POOL = False  # pairwise graph builds on GPSIMD (crashed a device once)
SKIP_SCATTER = False  # debug: replace local_scatter with memset (wrong output)
SC_BUFS = 3  # scan-buffer depth

ALU = mybir.AluOpType
AX = mybir.AxisListType
F32 = mybir.dt.float32
BF16 = mybir.dt.bfloat16
I16 = mybir.dt.int16
ACT = mybir.ActivationFunctionType
BIG = 1e30


def build_nc(repeat=1):
    nc = bacc.Bacc(trn_type="TRN2", target_bir_lowering=False)
    score_d = nc.declare_dram_parameter("score", [BLOC, F, P], F32, isOutput=False)
    out_d = nc.declare_dram_parameter("out", [BLOC, 1 + F * P], F32, isOutput=True)

    with tile.TileContext(nc) as tc:
        with (
            tc.tile_pool(name="consts", bufs=1) as cpool,
            tc.tile_pool(name="scan", bufs=2) as spool,
            tc.tile_pool(name="masks", bufs=2) as mpool,
            tc.tile_pool(name="grp", bufs=2) as gpool,
        ):
            # ---- constants ----
            prior17 = cpool.tile([128, P], BF16, name="prior17")
            nc.vector.memset(prior17[:], 0.0)
            p17v = prior17.rearrange("q (r c) -> q r c", r=N)
            nc.vector.memset(p17v[:, 4:14, 2:12], 17.0)
            # scatter data: w[f] = 17 - f  (17, 16, ..., 2)
            wvals = cpool.tile([128, F], BF16, name="wvals")
            nc.gpsimd.iota(
                wvals[:], pattern=[[-1, F]], base=17, channel_multiplier=0,
                allow_small_or_imprecise_dtypes=True,
            )
            # strict lower-triangular [e,f] mask (e < f), bcast over partitions
            ltri = cpool.tile([128, F, F], BF16, name="ltri")
            nc.vector.memset(ltri[:], 0.0)
            for e in range(F - 1):
                nc.vector.memset(ltri[:, e, e + 1 : F], 1.0)
            d1 = cpool.tile([128, G2 * F * S], BF16, name="d1")
            nc.vector.memset(d1[:], BIG)
            d1v = d1.rearrange("q (a f s) -> q a f s", a=G2, f=F)
            nc.vector.memset(d1v[:, :, :, 0:1], -BIG)

            for g in [g for _ in range(repeat) for g in range(NG)]:
                idxa = gpool.tile([128, F, GS], F32, tag="idxa", name="idxa")

                # ---- phase A: load / scan / popcounts, 2 tiles at a time ----
                for s in range(GS // G2):
                    r0 = (g * GS + s * G2) * 128
                    sc = spool.tile(
                        [128, G2 * F * S], F32, tag="sc", name="sc", bufs=SC_BUFS
                    )
                    scv = sc.rearrange("q (a f s) -> q a f s", a=G2, f=F)
                    nc.vector.memset(scv[:, :, :, 0:1], -BIG)
                    for j in range(G2):
                        nc.sync.dma_start(
                            out=scv[:, j, :, 1:S],
                            in_=score_d[r0 + j * 128 : r0 + (j + 1) * 128],
                        )
                    # in-place prefix-max scan with guard resets
                    nc.vector.tensor_tensor_scan(
                        sc[:], sc[:], d1[:], 0.0, ALU.max, ALU.min
                    )
                    # idx = #positions with prefix-max strictly below frame max
                    for j in range(G2):
                        t = s * G2 + j
                        for f in range(F):
                            if f < NF_DVE:
                                nc.vector.tensor_scalar(
                                    gpool.tile(
                                        [128, P], BF16, tag="vjunk",
                                        name="vjunk", bufs=2,
                                    ),
                                    scv[:, j, f, 1:S],
                                    scv[:, j, f, P : P + 1],
                                    None,
                                    ALU.is_lt,
                                    ALU.add,
                                    accum_out=idxa[:, f, t : t + 1],
                                )
                            else:
                                nc.scalar.activation(
                                    gpool.tile(
                                        [128, P], BF16, tag="sjunk",
                                        name="sjunk", bufs=2,
                                    ),
                                    scv[:, j, f, 1:S],
                                    ACT.Sign,
                                    bias=scv[:, j, f, P : P + 1],
                                    scale=-1.0,
                                    accum_out=idxa[:, f, t : t + 1],
                                )
                # ---- phase B: batched small compute for the whole group ----
                # r = round(idx/14 - 0.4643): value sits in [r-.46, r+.46],
                # so HW round-to-nearest int16 conversion recovers r exactly
                rq = gpool.tile([128, F, GS], F32, tag="rq", name="rq")
                nc.vector.tensor_scalar(
                    rq[:], idxa[:], 1.0 / 14.0, -0.4643, ALU.mult, ALU.add
                )
                ri = gpool.tile([128, F, GS], I16, tag="ri", name="ri")
                nc.vector.tensor_copy(ri[:], rq[:])
                rr = gpool.tile([128, F, GS], F32, tag="rr", name="rr")
                nc.vector.tensor_copy(rr[:], ri[:])
                cc = gpool.tile([128, F, GS], F32, tag="cc", name="cc")
                nc.vector.scalar_tensor_tensor(
                    cc[:], rr[:], -14.0, idxa[:], ALU.mult, ALU.add
                )
                vv = gpool.tile([128, F, GS], F32, tag="vv", name="vv")
                nc.vector.scalar_tensor_tensor(
                    vv[:], rr[:], 16.0, cc[:], ALU.mult, ALU.add
                )
                vb = gpool.tile([128, F, GS], BF16, tag="vb", name="vb")
                nc.vector.tensor_copy(vb[:], vv[:])

                # pairwise grid: dv[e,f,t] = v_e - v_f ; gg = adjacency
                ge = nc.gpsimd if GRAPH_ON_POOL else nc.vector
                dv = gpool.tile([128, F, F, GS], BF16, tag="dv", name="dv", bufs=1)
                ge.tensor_tensor(
                    dv[:],
                    vb.unsqueeze(2).broadcast_to([128, F, F, GS]),
                    vb.unsqueeze(1).broadcast_to([128, F, F, GS]),
                    ALU.subtract,
                )
                sq = gpool.tile([128, F, F, GS], BF16, tag="sq", name="sq", bufs=1)
                ge.tensor_tensor(sq[:], dv[:], dv[:], ALU.mult)
                g1 = gpool.tile([128, F, F, GS], BF16, tag="g1", name="g1", bufs=1)
                nc.vector.tensor_scalar(g1[:], sq[:], 1.0, None, ALU.is_equal)
                g16 = gpool.tile(
                    [128, F, F, GS], BF16, tag="g16", name="g16", bufs=1
                )
                nc.vector.tensor_scalar(g16[:], sq[:], 256.0, None, ALU.is_equal)
                gg = gpool.tile([128, F, F, GS], BF16, tag="gg", name="gg", bufs=1)
                ge.tensor_tensor(gg[:], g1[:], g16[:], ALU.add)
                # same-cell (for dedup): se[e,f,t] = (v_e == v_f) & (e < f)
                se = gpool.tile([128, F, F, GS], BF16, tag="se", name="se", bufs=1)
                nc.vector.scalar_tensor_tensor(
                    se[:],
                    sq[:],
                    0.0,
                    ltri.unsqueeze(3).broadcast_to([128, F, F, GS]),
                    ALU.is_equal,
                    ALU.mult,
                )

                # A = (r>=3 & 2<=c<=11) | (r>=4 & 1<=c<=12)
                u3 = gpool.tile([128, F, GS], BF16, tag="u3", name="u3")
                nc.vector.tensor_scalar(u3[:], rr[:], 3.0, None, ALU.is_ge)
                u4 = gpool.tile([128, F, GS], BF16, tag="u4", name="u4")
                nc.vector.tensor_scalar(u4[:], rr[:], 4.0, None, ALU.is_ge)
                cm2 = gpool.tile([128, F, GS], F32, tag="cm2", name="cm2")
                nc.vector.tensor_scalar(cm2[:], cc[:], 2.0, None, ALU.subtract)
                q1 = gpool.tile([128, F, GS], F32, tag="q1", name="q1")
                nc.vector.scalar_tensor_tensor(
                    q1[:], cc[:], -11.0, cm2[:], ALU.add, ALU.mult
                )
                b1 = gpool.tile([128, F, GS], BF16, tag="b1", name="b1")
                nc.vector.tensor_scalar(b1[:], q1[:], 0.0, None, ALU.is_le)
                cm1 = gpool.tile([128, F, GS], F32, tag="cm1", name="cm1")
                nc.vector.tensor_scalar(cm1[:], cc[:], 1.0, None, ALU.subtract)
                q2 = gpool.tile([128, F, GS], F32, tag="q2", name="q2")
                nc.vector.scalar_tensor_tensor(
                    q2[:], cc[:], -12.0, cm1[:], ALU.add, ALU.mult
                )
                b2 = gpool.tile([128, F, GS], BF16, tag="b2", name="b2")
                nc.vector.tensor_scalar(b2[:], q2[:], 0.0, None, ALU.is_le)
                t1 = gpool.tile([128, F, GS], BF16, tag="t1", name="t1")
                nc.vector.tensor_tensor(t1[:], u3[:], b1[:], ALU.logical_and)
                t2 = gpool.tile([128, F, GS], BF16, tag="t2", name="t2")
                nc.vector.tensor_tensor(t2[:], u4[:], b2[:], ALU.logical_and)
                aa = gpool.tile([128, F, GS], BF16, tag="aa", name="aa")
                nc.vector.tensor_tensor(aa[:], t1[:], t2[:], ALU.logical_or)

                # sequential added-recurrence:
                # added[f] = max(A[f], max_e added[e]*G[e,f])
                added = gpool.tile([128, F, GS], BF16, tag="added", name="added")
                nc.vector.memset(added[:], 0.0)
                t16 = gpool.tile([128, F, GS], BF16, tag="t16", name="t16")
                mx = gpool.tile([128, GS], F32, tag="mx", name="mx")
                for f in range(F):
                    nc.vector.tensor_tensor(
                        t16[:], added[:], gg[:, :, f, :], ALU.mult
                    )
                    t16v = t16.rearrange("q e t -> q t e")
                    nc.vector.tensor_reduce(mx[:], t16v, axis=AX.X, op=ALU.max)
                    nc.vector.tensor_tensor(
                        added[:, f, :], mx[:], aa[:, f, :], ALU.max
                    )

                # first-hit dedup: hb[f] = max_e added[e]*se[e,f]; fh = added & !hb
                hbt = gpool.tile(
                    [128, F, F, GS], BF16, tag="hbt", name="hbt", bufs=1
                )
                ge.tensor_tensor(
                    hbt[:],
                    added.unsqueeze(2).broadcast_to([128, F, F, GS]),
                    se[:],
                    ALU.mult,
                )
                hb = gpool.tile([128, F, GS], BF16, tag="hb", name="hb")
                hbtv = hbt.rearrange("q e f t -> q f t e")
                nc.vector.tensor_reduce(hb[:], hbtv, axis=AX.X, op=ALU.max)
                nhb = gpool.tile([128, F, GS], BF16, tag="nhb", name="nhb")
                nc.vector.tensor_scalar(nhb[:], hb[:], 0.0, None, ALU.is_equal)
                fh = gpool.tile([128, F, GS], BF16, tag="fh", name="fh")
                nc.vector.tensor_tensor(fh[:], added[:], nhb[:], ALU.mult)

                # scatter indices: idxs[f] = fh ? idx : -1, int16, t-major
                im0 = gpool.tile([128, F, GS], F32, tag="im0", name="im0")
                nc.vector.scalar_tensor_tensor(
                    im0[:], idxa[:], 1.0, fh[:], ALU.add, ALU.mult
                )
                idxm = gpool.tile([128, F, GS], F32, tag="idxm", name="idxm")
                nc.vector.tensor_scalar(idxm[:], im0[:], 1.0, None, ALU.subtract)
                idxs16 = gpool.tile([128, GS, F], I16, tag="idxs16", name="idxs16")
                nc.vector.tensor_copy(
                    idxs16[:], idxm.rearrange("q f t -> q t f")
                )

                # ---- phase C: scatter cov, compare-threshold, store 4 tiles ----
                for h in range(GS // G4):
                    r0 = (g * GS + h * G4) * 128
                    cov = gpool.tile([128, G4, P], BF16, tag="cov", name="cov")
                    for j in range(G4):
                        k = h * G4 + j
                        if SKIP_SCATTER:
                            nc.vector.memset(cov[:, j, :], 0.0)
                        else:
                            nc.gpsimd.local_scatter(
                                cov[:, j, :],
                                wvals[:],
                                idxs16[:, k, :],
                                channels=128,
                                num_elems=P,
                                num_idxs=F,
                            )
                    covm = gpool.tile([128, G4, P], BF16, tag="covm", name="covm")
                    nc.vector.tensor_tensor(
                        covm[:],
                        cov[:],
                        prior17.unsqueeze(1).broadcast_to([128, G4, P]),
                        ALU.max,
                    )
                    out_t = mpool.tile(
                        [128, G4, 1 + F * P], F32, tag="out_t", name="out_t"
                    )
                    nc.vector.memset(out_t[:, :, 0:1], 1.0)
                    for f in range(F):
                        nc.vector.tensor_scalar(
                            out_t[:, :, 1 + f * P : 1 + (f + 1) * P],
                            covm[:],
                            float(17 - f),
                            None,
                            ALU.is_ge,
                        )
                    nc.sync.dma_start(
                        out=out_d[r0 : r0 + G4 * 128].rearrange(
                            "(a p) w -> p a w", a=G4
                        ),
                        in_=out_t[:],
                    )

    nc.compile()
    return nc


_ncs = {}


def _get_nc(repeat=1):
    if repeat not in _ncs:
        _ncs[repeat] = build_nc(repeat)
    return _ncs[repeat]


def kernel(score, topn=196):
    score = np.ascontiguousarray(np.asarray(score, dtype=np.float32)).reshape(B, F, P)
    nc = _get_nc()
    in_maps = [
        {"score": score[i * BLOC : (i + 1) * BLOC]} for i in range(NCORES)
    ]
    res = run_bass_kernel_spmd(nc, in_maps, list(range(NCORES)))
    out = np.concatenate([res.results[i]["out"] for i in range(NCORES)], axis=0)
    return out

